# revision 1
# baseline (speedup 1.0000x reference)
"""Trainium2 Bass kernel for nn_Attention (linear attention w/ cubed feature map).

Math (per batch b):
  q = relu(in1 @ W.T + pos) / s ;  k = relu(in2 + pos) / s ;  s = softplus(scale_p)
  qf = (||q||/||q^3||) * q^3    ;  kf = (||k||/||k^3||) * k^3
  kv[h] = (1/N) * kf_h.T @ v_h  (v = in2),  per head h (32-dim blocks)
  out = sigmoid(q_f @ blockdiag(kv)) * in1

Distribution: sequence-parallel over N across 8 cores. Only the block-diagonal
[B,8,32,32] of kv is needed, so the per-head kv matmuls are emitted as [32,32]
blocks that land compactly in PSUM; the tiny block tensor is AllReduce'd per
half (2 batches), overlapping the q-side work.

Host pre-compute (all bf16): in2ps=(in2+pos)/s (k-side pre-activation, kills
the on-device scale+add), in2s=in2/s (v with 1/s folded; the s[e] factor is
re-applied for free via the sigmoid's per-partition scale AP), in1 transposed,
W/pos pre-scaled. pos enters the q-side projection via an identity matmul that
seeds the PSUM accumulator (no separate DVE add).

Engine budget (TimelineSim, per core): DVE ~100us is the bottleneck (relu+
square+cube chains at bf16 2x/4x rates), ACT ~86 (s3/w2 squares + sigmoid),
Pool ~45 (wq), PE ~55, DMA ~58. k-side row sums ride scalar_tensor_tensor
accum; q-side row sums are one-hot sel matmuls on PE. The per-row ratio is
broadcast across partitions with a DRAM-bounce stride-0 DMA.
"""

import numpy as np

B, N, D, H = 4, 16384, 256, 8
NCORES = 8
NS = N // NCORES          # 2048 positions per core
PK = 4                    # k-side row-tiles per pack
NPK = (NS // 128) // PK   # packs per batch = 4
ST = 512                  # q-side supertile rows
NST = NS // ST            # supertiles per batch = 4

DEFAULT_CFG = dict(
    sim=False,       # single-core variant w/o collective (TimelineSim only)
    a1_f32=False,    # ship in1t as f32 + f32r proj (accuracy fallback)
    asw=0,           # of 16 A-tiles, how many use the y+ACT path (DVE<->ACT balance)
    ot_pool=1,       # of 32 ot-muls, how many go to Pool instead of DVE
    wq_pool=24,      # of 32 wq-muls, how many go to Pool instead of DVE
    q2_pool=1,      # of 32 q2-muls, how many go to Pool instead of DVE
    q2_act=10,       # of 32 q2-muls (after the Pool ones), how many on ACT
    q3_pool=2,       # of 32 q3-muls, how many go to Pool
    vp_act=0,        # of 64 vp ratio-scales, how many on ACT (Copy w/ scale)
    w2_dve=17,        # of 32 w2-squares, how many (from the end) go to DVE
    w2_pool=0,       # of 32 w2-squares, how many (from the start) go to Pool
    k3_pool=0,       # of 16 A-tiles whose k3 mul runs on Pool
    rbs_dma=True,    # broadcast rat4 via DRAM-bounce DMA instead of Pool
    y_act=13,        # of 32 y-relus (psq evac), how many run on ACT
    s3_dve=2,        # of 16 A-tiles whose s3 row-sums run on DVE (stt)
    s3_sl=5,         # extra s3 SLICES (beyond s3_dve tiles) on DVE
    kvf_n=24,        # of 32 kvf block copies, how many on ACT (rest DVE)
    rbs_pb=0,        # of 4 batches (from the end) using partition_broadcast rbs
    ot_dma=False,    # sigmoid*in1 via DMA accum-mult (SWDGE too slow; keep off)
    y_act_end=False, # pick ACT-routed y units from the end of emission order
    kvf_act=True,    # kvf block copies on ACT instead of DVE
    dma_act=0,       # of 16 in2x loads, how many issue via the ACT DGE queue
    a1_bufs=16, y_bufs=16, q3_bufs=10, wq_bufs=10,
)

_BUILT = {}


def build(cfg=None):
    cfg = dict(DEFAULT_CFG, **(cfg or {}))
    key = tuple(sorted(cfg.items()))
    if key in _BUILT:
        return _BUILT[key]

    import concourse.bacc as bacc
    import concourse.mybir as mybir
    import concourse.tile as tile

    f32 = mybir.dt.float32
    f32r = mybir.dt.float32r
    bf16 = mybir.dt.bfloat16
    a1dt = f32 if cfg["a1_f32"] else bf16
    AF = mybir.ActivationFunctionType
    ALU = mybir.AluOpType

    nc = bacc.Bacc("TRN2", target_bir_lowering=False, debug=False,
                   num_devices=(1 if cfg["sim"] else NCORES))

    in2x_d = nc.dram_tensor("in2x", [B, NS, 2, D], bf16, kind="ExternalInput")
    in1t_d = nc.dram_tensor("in1t", [B, D, NS], a1dt, kind="ExternalInput")
    post_d = nc.dram_tensor("post", [D, NS], bf16, kind="ExternalInput")
    wt_d = nc.dram_tensor("wt", [D, D], a1dt, kind="ExternalInput")
    sel_d = nc.dram_tensor("sel", [128, NST * 4], bf16, kind="ExternalInput")
    sj_d = nc.dram_tensor("sj", [128, 2], f32, kind="ExternalInput")
    iden_d = nc.dram_tensor("iden", [128, 128], bf16, kind="ExternalInput")
    outt_d = nc.dram_tensor("outt", [B, D, NS], bf16, kind="ExternalOutput")

    in2x_r = in2x_d.ap().rearrange("b (pk t p) x f -> b pk p x t f",
                                   pk=NPK, t=PK, p=128)
    in1t_r = in1t_d.ap().rearrange("b (c p) r -> b c p r", p=128)
    post_r = post_d.ap().rearrange("(c p) r -> c p r", p=128)
    wt_r = wt_d.ap().rearrange("(c p) e -> c p e", p=128)
    outt_r = outt_d.ap().rearrange("b (c p) r -> b c p r", p=128)

    with tile.TileContext(nc) as tc:
        with (
            tc.tile_pool(name="const", bufs=1) as constp,
            tc.tile_pool(name="dram", bufs=1, space="DRAM") as dram,
            tc.tile_pool(name="ka", bufs=3) as kap,      # A-side streams
            tc.tile_pool(name="kb", bufs=3) as kbp,      # A-side mids
            tc.tile_pool(name="ksml", bufs=4) as ksml,   # A-side per-row smalls
            tc.tile_pool(name="qa", bufs=2) as qap,      # B-side persisted (bufs per-tag below)
            tc.tile_pool(name="qb", bufs=3) as qbp,      # B-side transients
            tc.tile_pool(name="qs", bufs=2) as qsp,      # B-side [4,ST] smalls
            tc.tile_pool(name="kvps", bufs=2, space="PSUM") as kvpsp,
            tc.tile_pool(name="psq", bufs=2, space="PSUM") as psqp,
            tc.tile_pool(name="psel", bufs=2, space="PSUM") as pselp,
            tc.tile_pool(name="px", bufs=2, space="PSUM") as pxp,
        ):
            # ---- resident constants ----
            wt_sb = constp.tile([128, 2, D], a1dt, tag="wt")
            post_sb = constp.tile([128, 2, NS], bf16, tag="post")
            sel_sb = constp.tile([128, NST * 4], bf16, tag="sel")
            sj_sb = constp.tile([128, 2], f32, tag="sj")
            iden_sb = constp.tile([128, 128], bf16, tag="iden")
            for c in range(2):
                nc.sync.dma_start(out=wt_sb[:, c, :], in_=wt_r[c])
            nc.sync.dma_start(out=sj_sb[:], in_=sj_d.ap())
            nc.sync.dma_start(out=iden_sb[:], in_=iden_d.ap())

            def load_late_consts(step):
                # post arrives in st-sized chunks so the first B-pre isn't
                # stuck behind one fat const DMA; sel comes with chunk 0.
                if step == 0:
                    nc.sync.dma_start(out=sel_sb[:], in_=sel_d.ap())
                sl = slice(step * ST, (step + 1) * ST)
                nc.sync.dma_start(out=post_sb[:, :, sl],
                                  in_=post_r[:, :, sl].rearrange("c p r -> p c r"))

            # kv result tiles: bf16, zeroed once; AllReduce'd blocks land in
            # the block-diagonal slots, the rest stays zero forever.
            kvf = []
            for b in range(B):
                kv_b = constp.tile([128, 2, D], bf16, tag=f"kvf{b}")
                nc.gpsimd.memset(kv_b[:], 0.0)
                kvf.append(kv_b)

            cc_in = dram.tile([B, 8, 32, 32], f32)
            cc_out = dram.tile([B, 8, 32, 32], f32)
            rat_dr = dram.tile([B, 4, ST], bf16)

            # ---------------- emission helpers ----------------
            kv_ps = {}
            kv_ps_half = {}
            k3c = [0]
            s3c_cnt = [0]
            kvfc = [0]
            ai_dma = [0]

            def a_tile(b, pk, use_act):
                i2x = kap.tile([128, 2, PK, D], bf16, tag="i2x")
                eng = nc.scalar if ai_dma[0] < cfg["dma_act"] else nc.sync
                eng.dma_start(out=i2x[:], in_=in2x_r[b, pk])
                ai_dma[0] += 1
                tps = i2x[:, 0]
                i2s = i2x[:, 1]
                s1c = ksml.tile([128, PK], f32, tag="s1c")
                s3c = ksml.tile([128, PK], f32, tag="s3c")
                k2 = kbp.tile([128, PK, D], bf16, tag="k2")
                k3 = kbp.tile([128, PK, D], bf16, tag="k3")
                if use_act:
                    # y on DVE (4x), squares+rowsums on ACT
                    y = kbp.tile([128, PK, D], bf16, tag="ya")
                    nc.vector.tensor_scalar_max(y[:], tps[:], 0.0)
                    for t in range(PK):
                        nc.scalar.activation(k2[:, t, :], y[:, t, :], AF.Square,
                                             accum_out=s1c[:, t:t + 1])
                    nc.vector.tensor_mul(k3[:], k2[:], y[:])
                else:
                    # fused relu*x(+rowsum) on DVE
                    for t in range(PK):
                        nc.vector.scalar_tensor_tensor(
                            out=k2[:, t, :], in0=tps[:, t, :], scalar=0.0,
                            in1=tps[:, t, :], op0=ALU.max, op1=ALU.mult,
                            accum_out=s1c[:, t:t + 1])
                    if k3c[0] < cfg["k3_pool"]:
                        nc.gpsimd.tensor_mul(k3[:], k2[:], tps[:])
                    else:
                        nc.vector.tensor_mul(k3[:], k2[:], tps[:])
                    k3c[0] += 1
                k6 = kbp.tile([128, PK, D], bf16, tag="k6")
                for t in range(PK):
                    if s3c_cnt[0] < cfg["s3_dve"] * PK + cfg["s3_sl"]:
                        nc.vector.scalar_tensor_tensor(
                            out=k6[:, t, :], in0=k3[:, t, :], scalar=0.0,
                            in1=k3[:, t, :], op0=ALU.max, op1=ALU.mult,
                            accum_out=s3c[:, t:t + 1])
                    else:
                        nc.scalar.activation(k6[:, t, :], k3[:, t, :],
                                             AF.Square,
                                             accum_out=s3c[:, t:t + 1])
                    s3c_cnt[0] += 1
                rec = ksml.tile([128, PK], f32, tag="rec")
                nc.vector.reciprocal(rec[:], s3c[:])
                rr = ksml.tile([128, PK], f32, tag="rr")
                nc.vector.tensor_mul(rr[:], s1c[:], rec[:])
                rat = ksml.tile([128, PK], f32, tag="rat")
                nc.scalar.activation(rat[:], rr[:], AF.Sqrt)
                vp = kbp.tile([128, PK, D], bf16, tag="vp")
                for t in range(PK):
                    if vpc[0] < cfg["vp_act"]:
                        nc.scalar.mul(vp[:, t, :], i2s[:, t, :],
                                      rat[:, t:t + 1])
                    else:
                        nc.vector.tensor_scalar_mul(
                            vp[:, t, :], i2s[:, t, :], rat[:, t:t + 1])
                    vpc[0] += 1
                # per-head [32,32] blocks — only the block-diagonal is needed,
                # and the blocks land compactly in [32, bi, h, 32] psum.
                for t in range(PK):
                    for h in range(H):
                        nc.tensor.matmul(
                            kv_ps[b][:, h, :],
                            lhsT=k3[:, t, 32 * h:32 * (h + 1)],
                            rhs=vp[:, t, 32 * h:32 * (h + 1)],
                            start=(pk == 0 and t == 0),
                            stop=(pk == NPK - 1 and t == PK - 1))

            a1s = {}   # (b, st) -> [a1_c0, a1_c1]
            ys = {}    # (b, st) -> [y_j0, y_j1]

            def b_pre(b, st):
                a1 = qap.tile([128, 2, ST], a1dt, tag="a1",
                              name=f"a1_{b}_{st}", bufs=cfg["a1_bufs"])
                nc.sync.dma_start(
                    out=a1[:],
                    in_=in1t_r[b, :, :, st * ST:(st + 1) * ST].rearrange(
                        "c p r -> p c r"))
                a1s[(b, st)] = a1
                y = qap.tile([128, 2, ST], bf16, tag="y",
                             name=f"y_{b}_{st}", bufs=cfg["y_bufs"])
                for j in range(2):
                    psq = psqp.tile([128, ST], f32, tag="psq")
                    nc.tensor.matmul(
                        psq[:], lhsT=iden_sb[:],
                        rhs=post_sb[:, j, st * ST:(st + 1) * ST],
                        start=True, stop=False)
                    for c in range(2):
                        nc.tensor.matmul(
                            psq[:],
                            lhsT=(wt_sb[:, c, j * 128:(j + 1) * 128].bitcast(f32r)
                                  if cfg["a1_f32"] else
                                  wt_sb[:, c, j * 128:(j + 1) * 128]),
                            rhs=(a1[:, c, :].bitcast(f32r) if cfg["a1_f32"]
                                 else a1[:, c, :]),
                            start=False, stop=(c == 1))
                    pick_act = (yc[0] >= 32 - cfg["y_act"]
                                if cfg.get("y_act_end") else
                                yc[0] < cfg["y_act"])
                    if pick_act:
                        nc.scalar.activation(y[:, j, :], psq[:], AF.Relu)
                    else:
                        nc.vector.tensor_scalar_max(y[:, j, :], psq[:], 0.0)
                    yc[0] += 1
                ys[(b, st)] = y

            q3s = {}   # (b, st) -> [q3_j0, q3_j1]
            ps14s, ps34s = {}, {}
            w2c = [0]
            q3c = [0]
            vpc = [0]
            q2c = [0]
            yc = [0]

            def b_mid(b, st):
                if st == 0:
                    psel = pselp.tile([36, ST], f32, tag="psel",
                                      name=f"psel_{b}")
                    ps14s[b] = psel[0:4, :]
                    ps34s[b] = psel[32:36, :]
                y = ys[(b, st)]
                q3 = qap.tile([128, 2, ST], bf16, tag="q3",
                              name=f"q3_{b}_{st}", bufs=cfg["q3_bufs"])
                for j in range(2):
                    q2 = qbp.tile([128, ST], bf16, tag="q2")
                    if q2c[0] < cfg["q2_pool"]:
                        nc.gpsimd.tensor_mul(q2[:], y[:, j, :], y[:, j, :])
                    elif q2c[0] < cfg["q2_pool"] + cfg["q2_act"]:
                        nc.scalar.activation(q2[:], y[:, j, :], AF.Square)
                    else:
                        nc.vector.tensor_mul(q2[:], y[:, j, :], y[:, j, :])
                    q2c[0] += 1
                    nc.tensor.matmul(
                        ps14s[b],
                        lhsT=sel_sb[:, st * 4:(st + 1) * 4], rhs=q2[:],
                        start=(st == 0 and j == 0),
                        stop=(st == NST - 1 and j == 1))
                    if q3c[0] < cfg["q3_pool"]:
                        nc.gpsimd.tensor_mul(q3[:, j, :], q2[:], y[:, j, :])
                    else:
                        nc.vector.tensor_mul(q3[:, j, :], q2[:], y[:, j, :])
                    q3c[0] += 1
                    w2 = qbp.tile([128, ST], bf16, tag="w2")
                    if w2c[0] >= 32 - cfg["w2_dve"]:
                        nc.vector.tensor_mul(w2[:], q3[:, j, :], q3[:, j, :])
                    elif w2c[0] < cfg["w2_pool"]:
                        nc.gpsimd.tensor_mul(w2[:], q3[:, j, :], q3[:, j, :])
                    else:
                        nc.scalar.activation(w2[:], q3[:, j, :], AF.Square)
                    w2c[0] += 1
                    nc.tensor.matmul(
                        ps34s[b],
                        lhsT=sel_sb[:, st * 4:(st + 1) * 4], rhs=w2[:],
                        start=(st == 0 and j == 0),
                        stop=(st == NST - 1 and j == 1))
                q3s[(b, st)] = q3

            wqs = {}   # (b, st) -> [wq_j0, wq_j1]
            wqc = [0]

            def b_rat(b):
                rec4 = qsp.tile([4, ST], f32, tag="rec4")
                nc.vector.reciprocal(rec4[:], ps34s[b])
                rr4 = qsp.tile([4, ST], f32, tag="rr4")
                nc.vector.tensor_mul(rr4[:], ps14s[b], rec4[:])
                rat4 = qsp.tile([4, ST], bf16, tag="rat4")
                nc.scalar.activation(rat4[:], rr4[:], AF.Sqrt,
                                     scale=1.0 / float(N) ** 2)
                use_pb = b >= B - cfg["rbs_pb"]
                if not use_pb:
                    nc.sync.dma_start(out=rat_dr[b], in_=rat4[:])
                    rbs_b = qbp.tile([128, NST, ST], bf16, tag="rbs_b")
                    nc.sync.dma_start(
                        out=rbs_b[:],
                        in_=rat_dr[b:b + 1].broadcast_to((128, NST, ST)))
                for st in range(NST):
                    if not use_pb:
                        rbs = rbs_b[:, st, :]
                    else:
                        rbst = qbp.tile([128, ST], bf16, tag="rbs")
                        nc.gpsimd.partition_broadcast(rbst[:],
                                                      rat4[st:st + 1, :])
                        rbs = rbst[:]
                    wqj = []
                    for j in range(2):
                        wq = qap.tile([128, ST], bf16, tag=f"wq_{j}",
                                      name=f"wq_{b}_{st}_{j}",
                                      bufs=cfg["wq_bufs"])
                        if wqc[0] < cfg["wq_pool"]:
                            nc.gpsimd.tensor_mul(wq[:], q3s[(b, st)][:, j, :],
                                                 rbs)
                        else:
                            nc.vector.tensor_mul(wq[:], q3s[(b, st)][:, j, :],
                                                 rbs)
                        wqc[0] += 1
                        wqj.append(wq)
                    wqs[(b, st)] = wqj

            def kv_evac(half, bs):
                kv_sb = kbp.tile([32, 2, H, 32], f32, tag="kvsb",
                                 name=f"kvsb{half}")
                nc.scalar.copy(kv_sb[:], kv_ps_half[half][:])
                nc.sync.dma_start(
                    out=cc_in[bs[0]:bs[1] + 1].rearrange("x h p f -> p x h f"),
                    in_=kv_sb[:])

            def kv_load(half, bs):
                stage = qbp.tile([32, 2, 8, 32], f32, tag="kvstage",
                                 name=f"kvstage{half}")
                nc.sync.dma_start(
                    out=stage[:],
                    in_=cc_out[bs[0]:bs[1] + 1].rearrange("x h p f -> p x h f"))
                for bi, b in enumerate(bs):
                    for c in range(2):
                        for g in range(4):
                            h = 4 * c + g
                            if cfg["kvf_act"] and kvfc[0] < cfg["kvf_n"]:
                                nc.scalar.copy(
                                    kvf[b][32 * g:32 * (g + 1), c,
                                           32 * h:32 * (h + 1)],
                                    stage[:, bi, h, :])
                            else:
                                nc.vector.tensor_copy(
                                    kvf[b][32 * g:32 * (g + 1), c,
                                           32 * h:32 * (h + 1)],
                                    stage[:, bi, h, :])
                            kvfc[0] += 1

            otc = [0]

            def b_tail(b, st):
                ot = qbp.tile([128, 2, ST], bf16, tag="ot")
                for j in range(2):
                    px = pxp.tile([128, ST], f32, tag="px")
                    for c in range(2):
                        nc.tensor.matmul(
                            px[:],
                            lhsT=kvf[b][:, c, j * 128:(j + 1) * 128],
                            rhs=wqs[(b, st)][j][:],
                            start=(c == 0), stop=(c == 1))
                    if cfg["ot_dma"]:
                        nc.scalar.activation(ot[:, j, :], px[:], AF.Sigmoid,
                                             scale=sj_sb[:, j:j + 1])
                        continue
                    sg = qbp.tile([128, ST], bf16, tag="sg")
                    nc.scalar.activation(sg[:], px[:], AF.Sigmoid,
                                         scale=sj_sb[:, j:j + 1])
                    a1j = a1s[(b, st)][:, j, :]
                    if cfg["a1_f32"]:
                        a1j = a1j.bitcast(f32)
                    if otc[0] < cfg["ot_pool"]:
                        nc.gpsimd.tensor_mul(ot[:, j, :], sg[:], a1j)
                    else:
                        nc.vector.tensor_mul(ot[:, j, :], sg[:], a1j)
                    otc[0] += 1
                eng = nc.gpsimd if cfg["ot_dma"] else nc.sync
                eng.dma_start(
                    out=outt_r[b, :, :, st * ST:(st + 1) * ST].rearrange(
                        "c p r -> p c r"),
                    in_=ot[:],
                    accum_op=(mybir.AluOpType.mult if cfg["ot_dma"]
                              else mybir.AluOpType.bypass))

            # ---------------- emission schedule ----------------
            # window 1: A-tiles (all 4 batches, halves) interleaved 1:1 with
            # B-pre units; per-half collectives right after each half's evac.
            pre_units = [(b, st) for b in range(B) for st in range(NST)]
            pre_it = iter(pre_units)
            n_act = cfg["asw"]
            ai = 0
            for half in range(2):
                bs = (2 * half, 2 * half + 1)
                kvh = kvpsp.tile([32, 2, H, 32], f32, tag="kv",
                                 name=f"kvps_h{half}")
                kv_ps_half[half] = kvh
                for bi, b in enumerate(bs):
                    kv_ps[b] = kvh[:, bi]
                for pk in range(NPK):
                    for b in bs:
                        # Bresenham spread of the n_act ACT-path tiles over 16
                        use_act = (ai * n_act) // 16 != ((ai + 1) * n_act) // 16
                        a_tile(b, pk, use_act)
                        if ai < NST:
                            load_late_consts(ai)
                        ai += 1
                        u = next(pre_it, None)
                        if u is not None:
                            b_pre(*u)
                kv_evac(half, bs)
                if cfg["sim"]:
                    nc.sync.dma_start(out=cc_out[bs[0]:bs[1] + 1],
                                      in_=cc_in[bs[0]:bs[1] + 1])
                else:
                    nc.gpsimd.collective_compute(
                        "AllReduce", mybir.AluOpType.add,
                        replica_groups=[list(range(NCORES))],
                        ins=[cc_in[bs[0]:bs[1] + 1].opt()],
                        outs=[cc_out[bs[0]:bs[1] + 1].opt()])

            # leftover B-pre units (if any)
            for u in pre_it:
                b_pre(*u)

            if cfg["ot_dma"]:
                for b in range(B):
                    nc.sync.dma_start(out=outt_d.ap()[b], in_=in1t_d.ap()[b])

            # window 2: mids+rats for half 0, then mids of half 1 finely
            # interleaved with tails of half 0 (per-st units, so no engine
            # queue head-of-line blocking), then tails of half 1 interleaved
            # b2/b3. sqrt<->sigmoid ACT table switches stay bounded.
            for b in (0, 1):
                for st in range(NST):
                    b_mid(b, st)
                b_rat(b)
            kv_load(0, (0, 1))
            t01 = [(b, st) for st in range(NST) for b in (0, 1)]
            t01_it = iter(t01)
            for b in (2, 3):
                for st in range(NST):
                    b_mid(b, st)
                    u = next(t01_it, None)
                    if u is not None:
                        b_tail(*u)
                b_rat(b)
            for u in t01_it:
                b_tail(*u)
            kv_load(1, (2, 3))
            for st in range(NST):
                for b in (2, 3):
                    b_tail(b, st)

    nc.compile()
    _BUILT[key] = nc
    return nc


def _prep_inputs(input1, input2, conv_w, pos_enc, scale_p, cfg=None):
    import ml_dtypes
    cfg = dict(DEFAULT_CFG, **(cfg or {}))
    bf16 = ml_dtypes.bfloat16
    a1dt = np.float32 if cfg["a1_f32"] else bf16

    s64 = np.logaddexp(scale_p.reshape(-1).astype(np.float64), 0.0)  # [256]
    inv_s = (1.0 / s64).astype(np.float64)
    pos64 = pos_enc[0].astype(np.float64)                            # [N, 256]

    in2x_full = np.empty((B, N, 2, 256), dtype=bf16)
    in2x_full[:, :, 1, :] = (input2.astype(np.float64) * inv_s).astype(bf16)
    in2x_full[:, :, 0, :] = ((input2.astype(np.float64) + pos64)
                             * inv_s).astype(bf16)
    in1t_full = np.ascontiguousarray(
        input1.transpose(0, 2, 1)).astype(a1dt)                      # [B, D, N]
    post_full = np.ascontiguousarray((pos64 * inv_s).T).astype(bf16)  # [D, N]
    wtp = np.ascontiguousarray(
        (conv_w.astype(np.float64) * inv_s[:, None]).T).astype(a1dt)  # [d, e]

    iden = np.eye(128, dtype=bf16)
    sel = np.zeros((128, NST * 4), dtype=bf16)
    for st in range(NST):
        sel[:, st * 4 + st] = 1
    sj = np.ascontiguousarray(
        s64.astype(np.float32).reshape(2, 128).T)                    # [128, 2]

    in_maps = []
    for core in range(NCORES):
        sl = slice(core * NS, (core + 1) * NS)
        in_maps.append({
            "in2x": np.ascontiguousarray(in2x_full[:, sl]),
            "in1t": np.ascontiguousarray(in1t_full[:, :, sl]),
            "post": np.ascontiguousarray(post_full[:, sl]),
            "wt": wtp,
            "sel": sel,
            "sj": sj,
            "iden": iden,
        })
    return in_maps


def kernel(input1, input2, conv_w, pos_enc, scale_p, _cfg=None, _trace=False):
    from concourse import bass_utils
    nc = build(_cfg)
    in_maps = _prep_inputs(input1, input2, conv_w, pos_enc, scale_p, _cfg)
    res = bass_utils.run_bass_kernel_spmd(
        nc, in_maps, core_ids=list(range(NCORES)), trace=_trace)
    out = np.concatenate(
        [r["outt"].transpose(0, 2, 1).astype(np.float32) for r in res.results],
        axis=1)
    kernel._last_results = res
    return np.ascontiguousarray(out)

